# revision 1
# baseline (speedup 1.0000x reference)
"""Two-layer GCN encoder on 8 Trainium2 NeuronCores (Bass/Tile).

Math (per layer, PyG GCNConv):
    deg[d]  = |{edges s->d}| + 1 (self loop)        [graph structure]
    dinv    = deg ** -0.5
    hs      = (dinv * x) @ W                        [= dinv * (x @ W)]
    agg[d]  = sum_{s in N(d) + self} hs[s]
    h       = relu(dinv * agg + b)
    out     = concat([h1, h2], axis=1)

Sharding: dst nodes are split evenly across the 8 cores.  Each core
computes hs for its own node shard (dense matmul), the shards are
AllGather'ed into a replicated hs_full table in DRAM, and each core
pulls hs_full[src] for the edges pointing into its shard with batched
gather DMA (dma_gather, int16 indices over <=32768-row source windows).
Messages arrive in dst-sorted 128-edge tiles; a 0/1 selection matrix
(built on the vector engine from each tile's local dst slots) routes
each tile through one PE matmul that segment-sums messages into a PSUM
accumulator per 128-node output block.

Tile numbering: span (cfg.span dst blocks) -> source window -> dst
block -> tile.  One dma_gather call covers one (span, window) range so
its output tiles are contiguous.

Host-side work is limited to graph preprocessing: degree counts, edge
sorting/padding, index layout, dtype casts.  All O(E*F) and O(N*F*F)
floating point work runs on the NeuronCores.
"""

import os
from dataclasses import dataclass

import ml_dtypes
import numpy as np

from concourse import bacc, bass, mybir
import concourse.tile as tile
from concourse.bass_utils import run_bass_kernel_spmd
from concourse.tile_rust import add_dep_helper
from concourse.library_config import mlp

BF16 = ml_dtypes.bfloat16
F32 = mybir.dt.float32
BF = mybir.dt.bfloat16
I32 = mybir.dt.int32
I16 = mybir.dt.int16

P = 128      # partitions / feature dim / edges per tile
WROWS = 32768  # int16 index window


@dataclass(frozen=True)
class Cfg:
    n_nodes: int
    n_edges: int
    feat: int = 128
    n_cores: int = 8
    span: int = 7  # dst blocks per gather span

    @property
    def npc(self):  # nodes per core
        assert self.n_nodes % self.n_cores == 0
        return self.n_nodes // self.n_cores

    @property
    def nblk(self):  # 128-node output blocks per core
        return -(-self.npc // P)

    @property
    def npcp(self):  # padded nodes per core
        return self.nblk * P

    @property
    def nn(self):  # rows of the allgathered hs table
        return self.n_cores * self.npcp

    @property
    def nwin(self):
        return -(-self.nn // WROWS)


CFG = Cfg(n_nodes=100000, n_edges=1600000)


def _layout(cfg: Cfg, T_bw):
    """Static tile layout shared by host prep and program build.

    T_bw: [nblk, nwin] tiles per (dst block, source window).
    Returns (spans, call_ranges, block_tiles, TT):
      spans: list of (b0, b1)
      call_ranges[s][w] = (gt0, gt1) global tile range of call (s, w)
      block_tiles[b] = list of (gt0, gt1) global tile ranges of block b
      TT = total tiles
    """
    nblk, nwin = T_bw.shape
    spans = [(b0, min(b0 + cfg.span, nblk)) for b0 in range(0, nblk, cfg.span)]
    call_ranges = []
    block_tiles = [[] for _ in range(nblk)]
    gt = 0
    for b0, b1 in spans:
        cr = []
        for w in range(nwin):
            wt0 = gt
            for b in range(b0, b1):
                tb = int(T_bw[b, w])
                if tb:
                    block_tiles[b].append((gt, gt + tb))
                gt += tb
            cr.append((wt0, gt))
        call_ranges.append(cr)
    return spans, call_ranges, block_tiles, gt


# ---------------------------------------------------------------------------
# Host-side graph preprocessing (indices only, plus dtype casts)
# ---------------------------------------------------------------------------

def prep_inputs(cfg: Cfg, x, edge_index, W1, b1, W2, b2):
    n, npc, npcp, nblk, nwin = cfg.n_nodes, cfg.npc, cfg.npcp, cfg.nblk, cfg.nwin

    x = np.asarray(x, dtype=np.float32)
    src = np.asarray(edge_index[0], dtype=np.int64)
    dst = np.asarray(edge_index[1], dtype=np.int64)
    loops = np.arange(n, dtype=np.int64)
    src_all = np.concatenate([src, loops])
    dst_all = np.concatenate([dst, loops])

    deg = np.bincount(dst_all, minlength=n).astype(np.float64)
    dinv = (1.0 / np.sqrt(deg)).astype(np.float32)  # deg >= 1 via self loop

    # row of node v inside the allgathered hs table (shards are padded)
    hsrow_all = ((src_all // npc) * npcp + (src_all % npc)).astype(np.int64)
    core_of_dst = dst_all // npc

    per_core = []
    cnts = np.zeros((cfg.n_cores, nblk * nwin), dtype=np.int64)
    for c in range(cfg.n_cores):
        m = core_of_dst == c
        srows = hsrow_all[m]
        dloc = dst_all[m] - c * npc
        win = srows // WROWS
        key = (dloc >> 7) * nwin + win  # (block, window) group id
        order = np.argsort(key, kind="stable")
        srows, key = srows[order], key[order]
        slot = (dloc[order] & 127)
        cnts[c] = np.bincount(key, minlength=nblk * nwin)
        per_core.append((srows, key, slot))

    # tiles per (block, window): max over cores -> identical program
    T_bw = (-(-cnts // P)).max(axis=0).reshape(nblk, nwin)
    spans, call_ranges, block_tiles, TT = _layout(cfg, T_bw)

    # global tile base per (block, window) group, in the span->window order
    g_base = np.zeros(nblk * nwin, dtype=np.int64)
    for b in range(nblk):
        ranges = iter(block_tiles[b])
        for w in range(nwin):
            if T_bw[b, w]:
                gt0, _ = next(ranges)
                g_base[b * nwin + w] = gt0

    xs = x * dinv[:, None]  # fold dinv into the layer-1 matmul input

    in_maps = []
    for c in range(cfg.n_cores):
        srows, key, slot = per_core[c]
        start = np.concatenate([[0], np.cumsum(cnts[c])[:-1]])
        pos = np.arange(len(key)) - start[key]
        gtile = g_base[key] + (pos >> 7)
        gpart = pos & 127

        V = np.zeros((TT, P), np.int64)     # window-local source row per msg
        D = np.full((TT, P), -1.0, np.float32)  # local dst slot (-1 = dummy)
        V[gtile, gpart] = srows % WROWS
        D[gtile, gpart] = slot

        # idx16: per call (s, w) the columns [8*gt0, 8*gt1); within a call,
        # msg row j = (t - gt0)*128 + p lives at [16g + (j%16), gt0*8 + j//16]
        idx16 = np.zeros((P, TT * 8), np.int16)
        for s in range(len(spans)):
            for w in range(nwin):
                gt0, gt1 = call_ranges[s][w]
                if gt1 == gt0:
                    continue
                v = V[gt0:gt1, :].reshape(-1)  # j order: t-major, p minor
                blockv = v.reshape(-1, 16).T.astype(np.int16)  # [16, ncols]
                idx16[:, gt0 * 8:gt1 * 8] = np.tile(blockv, (8, 1))

        xT = np.zeros((P, npcp), np.float32)
        xT[:, :npc] = xs[c * npc:(c + 1) * npc].T
        dv = np.zeros(npcp, np.float32)
        dv[:npc] = dinv[c * npc:(c + 1) * npc]
        dinvT = np.ascontiguousarray(dv.reshape(nblk, P).T)

        in_maps.append(
            {
                "xT": xT.astype(BF16),
                "idx16": idx16,
                "dsel": np.ascontiguousarray(D.T).astype(BF16),
                "dinvT": dinvT,
                "w1": np.asarray(W1, np.float32).astype(BF16),
                "w2": np.asarray(W2, np.float32).astype(BF16),
                "bb1": np.broadcast_to(np.asarray(b1, np.float32), (P, cfg.feat)).copy(),
                "bb2": np.broadcast_to(np.asarray(b2, np.float32), (P, cfg.feat)).copy(),
                "iot": np.broadcast_to(np.arange(P, dtype=np.float32), (P, P)).copy().astype(BF16),
            }
        )
    return in_maps, T_bw


# ---------------------------------------------------------------------------
# Device program
# ---------------------------------------------------------------------------

def build_program(cfg: Cfg, T_bw):
    n_f = cfg.feat
    npc, npcp, nblk, nwin, nn = cfg.npc, cfg.npcp, cfg.nblk, cfg.nwin, cfg.nn
    spans, call_ranges, block_tiles, TT = _layout(cfg, T_bw)

    nc = bacc.Bacc("TRN2", target_bir_lowering=False, debug=False,
                   num_devices=cfg.n_cores)

    xT_d = nc.dram_tensor("xT", [P, npcp], BF, kind="ExternalInput")
    idx16_d = nc.dram_tensor("idx16", [P, TT * 8], I16, kind="ExternalInput")
    dsel_d = nc.dram_tensor("dsel", [P, TT], BF, kind="ExternalInput")
    dinvT_d = nc.dram_tensor("dinvT", [P, nblk], F32, kind="ExternalInput")
    w_d = [nc.dram_tensor("w1", [n_f, n_f], BF, kind="ExternalInput"),
           nc.dram_tensor("w2", [n_f, n_f], BF, kind="ExternalInput")]
    bb_d = [nc.dram_tensor("bb1", [P, n_f], F32, kind="ExternalInput"),
            nc.dram_tensor("bb2", [P, n_f], F32, kind="ExternalInput")]
    iot_d = nc.dram_tensor("iot", [P, P], BF, kind="ExternalInput")
    out_d = nc.dram_tensor("out", [npc, 2 * n_f], F32, kind="ExternalOutput")

    s2_sh = nc.dram_tensor("s2sh", [npcp, n_f], BF)  # dinv * h1 (layer-2 input)
    hs_sh = [nc.dram_tensor(f"hs{L}sh", [npcp, n_f], BF) for L in (1, 2)]
    hs_full = [nc.dram_tensor(f"hs{L}full", [nn, n_f], BF,
                              addr_space="Shared") for L in (1, 2)]
    groups = [list(range(cfg.n_cores))]

    with tile.TileContext(nc) as tc:
        with (
            tc.tile_pool(name="const", bufs=1) as cpool,
            tc.tile_pool(name="big", bufs=1) as bigpool,
            tc.tile_pool(name="xw", bufs=3) as xwpool,
            tc.tile_pool(name="idx", bufs=2) as idxpool,
            tc.tile_pool(name="msg", bufs=2) as msgpool,
            tc.tile_pool(name="sel", bufs=8) as selpool,
            tc.tile_pool(name="post", bufs=3) as postpool,
            tc.tile_pool(name="psxw", bufs=2, space="PSUM") as psxw,
            tc.tile_pool(name="psag", bufs=4, space="PSUM") as psag,
        ):
            nc.gpsimd.load_library(mlp)
            w_t, bb_t = [], []
            for L in (0, 1):
                wt = cpool.tile([n_f, n_f], BF, tag=f"w{L}", name=f"w{L}t")
                nc.sync.dma_start(out=wt[:], in_=w_d[L][:])
                w_t.append(wt)
                bt = cpool.tile([P, n_f], F32, tag=f"bb{L}", name=f"bb{L}t")
                nc.sync.dma_start(out=bt[:], in_=bb_d[L][:])
                bb_t.append(bt)
            iot_t = cpool.tile([P, P], BF, tag="iot", name="iot_t")
            nc.sync.dma_start(out=iot_t[:], in_=iot_d[:])
            dinvT_t = cpool.tile([P, nblk], F32, tag="dinvT", name="dinvT_t")
            nc.sync.dma_start(out=dinvT_t[:], in_=dinvT_d[:])

            xT_t = [bigpool.tile([P, npcp], BF, tag="xT1", name="xT1_t"),
                    bigpool.tile([P, npcp], BF, tag="xT2", name="xT2_t")]
            nc.sync.dma_start(out=xT_t[0][:], in_=xT_d[:])

            def xw_phase(L):
                """hs_sh[L] = (xT_t[L].T @ W_L) as bf16, node-major."""
                stores = []
                for t in range(nblk):
                    ps = psxw.tile([P, n_f], F32, tag="psxw", name="psxw_t")
                    nc.tensor.matmul(out=ps[:], lhsT=xT_t[L][:, t * P:(t + 1) * P],
                                     rhs=w_t[L][:], start=True, stop=True)
                    hsb = xwpool.tile([P, n_f], BF, tag="hsb", name="hsb_t")
                    nc.vector.tensor_copy(out=hsb[:], in_=ps[:])
                    stores.append(
                        nc.sync.dma_start(out=hs_sh[L][t * P:(t + 1) * P, :],
                                          in_=hsb[:]))
                return stores

            def allgather(L, stores):
                ag = nc.gpsimd.collective_compute(
                    "AllGather", mybir.AluOpType.bypass, replica_groups=groups,
                    ins=[hs_sh[L][:]], outs=[hs_full[L][:]])
                for s in stores:
                    add_dep_helper(ag.ins, s.ins, reason="allgather after hs stores")
                return ag

            STAGE = int(os.environ.get("GCN_STAGE", "9"))

            def agg_layer(L, ag):
                """Pull messages, segment-sum per 128-dst block, postprocess."""
                s2_stores = []
                for si, (b0, b1) in enumerate(spans):
                    t0 = call_ranges[si][0][0]
                    t1 = call_ranges[si][nwin - 1][1]
                    ts = t1 - t0
                    dsel_t = idxpool.tile([P, ts], BF, tag="dsel", name="dsel_t")
                    nc.sync.dma_start(out=dsel_t[:], in_=dsel_d[:, t0:t1])
                    msg = msgpool.tile([P, ts, n_f], BF, tag="msg", name="msg_t")
                    for w in range(nwin):
                        gt0, gt1 = call_ranges[si][w]
                        if gt1 == gt0:
                            continue
                        nidx = (gt1 - gt0) * P
                        it16 = idxpool.tile([P, (gt1 - gt0) * 8], I16,
                                            tag="idx16", name="it16_t")
                        nc.sync.dma_start(out=it16[:],
                                          in_=idx16_d[:, gt0 * 8:gt1 * 8])
                        wb = w * WROWS
                        wr = min(WROWS, nn - wb)
                        g = nc.gpsimd.dma_gather(
                            msg[:, gt0 - t0:gt1 - t0, :],
                            hs_full[L][wb:wb + wr, :], it16[:],
                            nidx, nidx, n_f, single_packet=False)
                        add_dep_helper(g.ins, ag.ins, reason="gather after ag")
                    if STAGE <= 3:
                        continue
                    for b in range(b0, b1):
                        # selection matrices per tile range, built just before
                        # their matmuls so DVE order matches PE consumption
                        nt = sum(g1 - g0 for g0, g1 in block_tiles[b])
                        ps = psag.tile([P, n_f], F32, tag="psag", name="psag_t")
                        k = 0
                        for g0, g1 in block_tiles[b]:
                            rn = g1 - g0
                            sel = selpool.tile([P, rn, P], BF, tag="sel",
                                               name="sel_t")
                            nc.vector.tensor_tensor(
                                out=sel[:],
                                in0=iot_t[:, None, :].to_broadcast([P, rn, P]),
                                in1=dsel_t[:, g0 - t0:g1 - t0, None]
                                    .to_broadcast([P, rn, P]),
                                op=mybir.AluOpType.is_equal)
                            if STAGE <= 4:
                                continue
                            for j in range(rn):
                                nc.tensor.matmul(out=ps[:],
                                                 lhsT=sel[:, j, :],
                                                 rhs=msg[:, g0 - t0 + j, :],
                                                 start=(k == 0),
                                                 stop=(k == nt - 1))
                                k += 1
                        if STAGE <= 4:
                            continue
                        if STAGE <= 5:
                            continue
                        # h = relu(dinv * agg + b)
                        t0f = postpool.tile([P, n_f], F32, tag="t0f", name="t0f_t")
                        nc.vector.tensor_scalar(
                            out=t0f[:], in0=ps[:], scalar1=dinvT_t[:, b:b + 1],
                            scalar2=None, op0=mybir.AluOpType.mult)
                        nc.vector.tensor_tensor(out=t0f[:], in0=t0f[:],
                                                in1=bb_t[L][:],
                                                op=mybir.AluOpType.add)
                        h_t = postpool.tile([P, n_f], F32, tag="hrelu", name="hrelu_t")
                        nc.scalar.activation(out=h_t[:], in_=t0f[:],
                                             func=mybir.ActivationFunctionType.Relu)
                        rows = min(P, npc - b * P)
                        nc.scalar.dma_start(
                            out=out_d[b * P:b * P + rows, L * n_f:(L + 1) * n_f],
                            in_=h_t[:rows, :])
                        if L == 0:
                            s2_t = postpool.tile([P, n_f], BF, tag="s2", name="s2_t")
                            nc.vector.tensor_scalar(
                                out=s2_t[:], in0=h_t[:],
                                scalar1=dinvT_t[:, b:b + 1], scalar2=None,
                                op0=mybir.AluOpType.mult)
                            s2_stores.append(
                                nc.scalar.dma_start(
                                    out=s2_sh[b * P:(b + 1) * P, :], in_=s2_t[:]))
                return s2_stores

            st1 = xw_phase(0)
            if STAGE >= 2:
                ag1 = allgather(0, st1)
            if STAGE >= 3:
                s2st = agg_layer(0, ag1)
            if STAGE >= 7:
                tr = nc.sync.dma_start_transpose(out=xT_t[1][:], in_=s2_sh[:])
                for s in s2st:
                    add_dep_helper(tr.ins, s.ins, reason="transpose after s2 stores")
            if STAGE >= 8:
                st2 = xw_phase(1)
                ag2 = allgather(1, st2)
                agg_layer(1, ag2)

    nc.compile()
    return nc


# ---------------------------------------------------------------------------
# Entry point
# ---------------------------------------------------------------------------

_CACHE: dict = {}


def _install_ntff_hook():
    """Wire the axon NTFF profiling hook that this image leaves unplugged.

    Harness-side instrumentation only; no-op when already present or
    when the pieces are missing."""
    try:
        from antenv.axon_hooks import get_axon_ntff_profile_hook  # noqa: F401
        return
    except ImportError:
        pass
    try:
        import sys
        import types

        if "/root/.axon_site" not in sys.path:
            sys.path.insert(0, "/root/.axon_site")
        from trn_agent_boot.trn_boot import _ntff_profile_via_ctypes

        hook = _ntff_profile_via_ctypes("/opt/axon/libaxon_pjrt.so")
        import antenv

        m = types.ModuleType("antenv.axon_hooks")
        m.get_axon_ntff_profile_hook = lambda: hook
        m.set_axon_ntff_profile_hook = lambda h: None
        sys.modules["antenv.axon_hooks"] = m
        antenv.axon_hooks = m
        import concourse.bass_utils as bu

        bu.upload_artifacts = lambda tmpdir: f"local:{tmpdir}"
    except Exception as e:  # degrade to no tracing
        print("ntff hook install failed:", e)


def run(cfg: Cfg, inputs: dict, trace: bool = False):
    if trace:
        _install_ntff_hook()
    in_maps, T_bw = prep_inputs(cfg, **inputs)
    key = (cfg, T_bw.tobytes())
    if key not in _CACHE:
        _CACHE[key] = build_program(cfg, T_bw)
    nc = _CACHE[key]
    res = run_bass_kernel_spmd(nc, in_maps, list(range(cfg.n_cores)), trace=trace)
    out = np.concatenate([res.results[c]["out"] for c in range(cfg.n_cores)], axis=0)
    return out, res


def kernel(**inputs) -> np.ndarray:
    trace = bool(os.environ.get("BASS_TRACE"))
    out, _ = run(CFG, inputs, trace=trace)
    return out

